# revision 50
# baseline (speedup 1.0000x reference)
"""GQA attention kernel for Trainium2, 8-core SPMD.

Sharding: core c = 2*b + g handles batch b (of 4) and head-group g (of 2):
8 of 16 q-heads, 2 of 4 kv-heads.  Each core computes its partial
out^T = (attn_out @ wo_g^T)^T in transposed space; the host adds the two
group partials per batch and transposes back.

v2.1 design (vs the fp32r 3-phase baseline):
  - all matmul operands bf16 (FWL weight loads, single x pass, half DMA)
  - x, Q, K, V, weights SBUF-resident; no Qd/Od DRAM round trips
  - DMAs emitted in first-use order so the PE starts within ~5us
  - fused attention + output projection in one qj-major loop
  - score tiles ki-paired: one exp per [128,2,512] PSUM region, den tree
    on [128,1024] pair tiles
  - softmax tail (den-reduce, 1/den, partition_broadcast, normalize)
    software-pipelined one head behind the s/PV matmuls so the in-order
    PE queue never stalls on DVE/Act work
  - output projection written to DRAM directly from PSUM (no Act copy)
Everything on-chip is in transposed orientation:
  Q^T/K^T: [head_dim(part), T]  scores^T: [kt(part), qt]  O^T: [d(part), qt]
RoPE is handled by permuting wq/wk rows on the host to an [evens | odds]
layout (scores are invariant to a shared d-permutation).
"""

import math
import numpy as np

B, T, C = 4, 2048, 2048
N_HEAD, N_KV_HEAD, HD = 16, 4, 128
N_CORES = 8
SCALE = 1.0 / math.sqrt(HD)

_PROG = {}
_LAST_IN_MAPS = None


def _build_program():
    from contextlib import ExitStack
    import concourse.bacc as bacc
    import concourse.mybir as mybir
    import concourse.tile as tile

    f32 = mybir.dt.float32
    bf16 = mybir.dt.bfloat16
    fp8 = mybir.dt.float8e4
    DR = mybir.MatmulPerfMode.DoubleRow
    Exp = mybir.ActivationFunctionType.Exp

    nc = bacc.Bacc(None, target_bir_lowering=False)
    xT = nc.declare_dram_parameter("xT", [C, T], bf16, isOutput=False)
    wqT = nc.declare_dram_parameter("wqT", [C, 1024], bf16, isOutput=False)
    wkT = nc.declare_dram_parameter("wkT", [C, 256], bf16, isOutput=False)
    wvT = nc.declare_dram_parameter("wvT", [C, 256], bf16, isOutput=False)
    woT = nc.declare_dram_parameter("woT", [1024, T], bf16, isOutput=False)
    cs2D = nc.declare_dram_parameter("cs2", [128, T], bf16, isOutput=False)
    sb2D = nc.declare_dram_parameter("sb2", [128, T], bf16, isOutput=False)
    out = nc.declare_dram_parameter("out", [C, T], f32, isOutput=True)

    with tile.TileContext(nc) as tc, nc.allow_low_precision(
        reason="bf16 operands validated end-to-end against 2e-2 rel-err gate"
    ), ExitStack() as top:
        consts = top.enter_context(tc.tile_pool(name="consts", bufs=1))
        cs2 = consts.tile([128, T], bf16)
        sb2 = consts.tile([128, T], bf16)
        ones_f = consts.tile([128, 1], bf16)
        nc.vector.memset(ones_f, 1.0)

        persist = top.enter_context(tc.tile_pool(name="persist", bufs=1))
        Q_sb = persist.tile([128, 8, T], bf16)
        K_sb = persist.tile([128, 2, T], bf16)
        V_sb = persist.tile([128, 16, 256], bf16)

        # ---- phase 1: QKV projections + RoPE, everything stays in SBUF ----
        with ExitStack() as ph1:
            xp = ph1.enter_context(tc.tile_pool(name="xp", bufs=1))
            x_sb = xp.tile([128, 16, T], bf16)
            wp = ph1.enter_context(tc.tile_pool(name="wp", bufs=1))
            wk_sb = wp.tile([128, 16, 256], bf16)
            wv_sb = wp.tile([128, 16, 256], bf16)
            wq_sb = wp.tile([128, 16, 1024], bf16)
            # Bulk loads all on the SP queue in PE-consumption order (the
            # DMA fabric is bandwidth-limited early on, so splitting queues
            # only reorders arrivals against the in-order consumer).
            nc.sync.dma_start(
                out=wk_sb, in_=wkT.rearrange("(n p) m -> p n m", p=128)
            )
            for ci in range(16):
                nc.sync.dma_start(
                    out=x_sb[:, ci, :],
                    in_=xT[ci * 128:(ci + 1) * 128, :],
                )
            nc.sync.dma_start(out=cs2, in_=cs2D[:])
            nc.sync.dma_start(out=sb2, in_=sb2D[:])
            nc.sync.dma_start(
                out=wv_sb, in_=wvT.rearrange("(n p) m -> p n m", p=128)
            )
            nc.sync.dma_start(
                out=wq_sb, in_=wqT.rearrange("(n p) m -> p n m", p=128)
            )

            raws = ph1.enter_context(tc.tile_pool(name="raws", bufs=4))
            tatb = ph1.enter_context(tc.tile_pool(name="tatb", bufs=4))
            pj_ps = ph1.enter_context(
                tc.tile_pool(name="pj_ps", bufs=3, space="PSUM")
            )

            def proj_half(w_sb, fsl, hb):
                """Project one 128-feature block over a 1024-token half.
                Returns the raw (pre-RoPE) bf16 SBUF tile [128, 1024]."""
                hsl = slice(hb * 1024, (hb + 1) * 1024)
                ps = pj_ps.tile([128, 2, 512], f32, tag="pj", name="pj")
                for ci in range(16):
                    for c2 in range(2):
                        t0 = hb * 1024 + c2 * 512
                        nc.tensor.matmul(
                            ps[:, c2, :],
                            w_sb[:, ci, fsl],
                            x_sb[:, ci, t0:t0 + 512],
                            start=(ci == 0), stop=(ci == 15),
                        )
                raw = raws.tile([128, 1024], bf16, tag="raw", name="raw")
                for c2 in range(2):
                    nc.vector.tensor_copy(
                        raw[:, c2 * 512:(c2 + 1) * 512], ps[:, c2, :]
                    )
                return raw, hsl

            def rope_half(raw, hsl, dest):
                """dest[:, hsl] = raw * cs2 + swap(raw) * sb2.
                The half-swap along the partition (head-dim) axis is done
                with two SBUF->SBUF DMAs instead of a permutation matmul."""
                swr = raws.tile([128, 1024], bf16, tag="swr", name="swr")
                nc.sync.dma_start(out=swr[0:64, :], in_=raw[64:128, :])
                nc.sync.dma_start(out=swr[64:128, :], in_=raw[0:64, :])
                ta = tatb.tile([128, 1024], bf16, tag="ta", name="ta")
                tb = tatb.tile([128, 1024], bf16, tag="tb", name="tb")
                nc.vector.tensor_mul(ta, raw, cs2[:, hsl])
                nc.vector.tensor_mul(tb, swr, sb2[:, hsl])
                nc.vector.tensor_add(dest, ta, tb)

            # K (2 kv heads, rope'd) first so attention deps resolve early
            for kv in range(2):
                for hb in range(2):
                    raw, hsl = proj_half(wk_sb, slice(kv * 128, (kv + 1) * 128), hb)
                    rope_half(raw, hsl, K_sb[:, kv, hsl])
            # V (2 kv heads = 2 d-chunks), transposed into [t, d] layout
            # via the DMA XBAR instead of PE transposes
            for dv in range(2):
                for hb in range(2):
                    raw, hsl = proj_half(wv_sb, slice(dv * 128, (dv + 1) * 128), hb)
                    for k8 in range(8):
                        ki = hb * 8 + k8
                        nc.sync.dma_start_transpose(
                            out=V_sb[:, ki, dv * 128:(dv + 1) * 128],
                            in_=raw[:, k8 * 128:(k8 + 1) * 128],
                        )
            # Q (8 heads, rope'd)
            for h in range(8):
                for hb in range(2):
                    raw, hsl = proj_half(
                        wq_sb, slice(h * 128, (h + 1) * 128), hb
                    )
                    rope_half(raw, hsl, Q_sb[:, h, hsl])

        # ---- phase 2: causal attention fused with output projection ----
        with ExitStack() as ph2:
            wop = ph2.enter_context(tc.tile_pool(name="wop", bufs=1))
            wo_sb = wop.tile([128, 8, T], bf16)
            nc.sync.dma_start(
                out=wo_sb, in_=woT.rearrange("(h p) e -> p h e", p=128)
            )
            p_pool = ph2.enter_context(tc.tile_pool(name="p_pool", bufs=20))
            dtree = ph2.enter_context(tc.tile_pool(name="dtree", bufs=12))
            dn128 = ph2.enter_context(tc.tile_pool(name="dn128", bufs=2))
            rcp = ph2.enter_context(tc.tile_pool(name="rcp", bufs=2))
            rbp = ph2.enter_context(tc.tile_pool(name="rbp", bufs=2))
            osb = ph2.enter_context(tc.tile_pool(name="osb", bufs=2))
            obuf = ph2.enter_context(tc.tile_pool(name="obuf", bufs=4))
            ps_s = ph2.enter_context(
                tc.tile_pool(name="ps_s", bufs=2, space="PSUM")
            )
            ps_o = ph2.enter_context(
                tc.tile_pool(name="ps_o", bufs=2, space="PSUM")
            )
            ps_m = ph2.enter_context(
                tc.tile_pool(name="ps_m", bufs=2, space="PSUM")
            )

            o_alls = {}

            def emit_body(h, qj):
                """s/exp/mask/PV matmuls for one (head, q-tile).
                Score tiles are processed in ki pairs: 2 s-matmuls into one
                [128,2,512] PSUM region, a single exp over both, then 2 PV
                accumulation matmuls.  Diagonal-tile matmuls are narrowed to
                the causally live columns; exp/select stay full-width so the
                skipped (stale) columns are forced to exact zero."""
                kv = h // 4
                nk = 4 * (qj + 1)
                o_ps = ps_o.tile([128, 512], f32, name="o_ps")
                pairs = []
                for kp in range(nk // 2):
                    s_ps = ps_s.tile([128, 2, 512], f32, name="s_ps")
                    for j in range(2):
                        ki = 2 * kp + j
                        lo = max(0, ki * 128 - qj * 512)
                        nc.tensor.matmul(
                            s_ps[:, j, lo:],
                            K_sb[:, kv, ki * 128:(ki + 1) * 128],
                            Q_sb[:, h, qj * 512 + lo:(qj + 1) * 512],
                        )
                    pp = p_pool.tile([128, 2, 512], bf16, tag="p", name="p")
                    # exp narrowed to causally-live columns of the pair; the
                    # full-width selects below zero everything to the left
                    lo0 = max(0, 2 * kp * 128 - qj * 512)
                    nc.scalar.activation(
                        pp[:, :, lo0:], s_ps[:, :, lo0:], Exp, scale=SCALE
                    )
                    for j in range(2):
                        ki = 2 * kp + j
                        if ki >= 4 * qj:
                            nc.gpsimd.affine_select(
                                out=pp[:, j, :], in_=pp[:, j, :],
                                pattern=[[1, 512]],
                                compare_op=mybir.AluOpType.is_ge, fill=0.0,
                                base=qj * 512 - ki * 128,
                                channel_multiplier=-1,
                            )
                    for j in range(2):
                        ki = 2 * kp + j
                        lo = max(0, ki * 128 - qj * 512)
                        nc.tensor.matmul(
                            o_ps[:, lo:],
                            V_sb[:, ki, kv * 128:(kv + 1) * 128],
                            pp[:, j, lo:],
                            start=(ki == 0), stop=(ki == nk - 1),
                        )
                    pairs.append(pp)
                return h, qj, pairs, o_ps

            def emit_tail(h, qj, pairs, o_ps):
                """den reduce + 1/den + normalize into O_all[qj][:, h, :]."""
                # pairwise bf16 tree over [128,1024] pair tiles; the first
                # level of long tails runs on the otherwise-idle Pool engine
                lvl = pairs
                first = True
                while len(lvl) > 1:
                    nxt = []
                    for i in range(0, len(lvl) - 1, 2):
                        t = dtree.tile([128, 2, 512], bf16, tag="dt", name="dt")
                        eng = nc.gpsimd if (first and len(lvl) >= 6 and
                                            i % 4 == 0) else nc.vector
                        eng.tensor_add(t, lvl[i], lvl[i + 1])
                        nxt.append(t)
                    if len(lvl) % 2:
                        nxt.append(lvl[-1])
                    lvl = nxt
                    first = False
                den = dn128.tile([128, 512], bf16, tag="dn", name="dn")
                nc.vector.tensor_add(den, lvl[0][:, 0, :], lvl[0][:, 1, :])
                den_ps = ps_m.tile([1, 512], f32, tag="m", name="dps")
                nc.tensor.matmul(den_ps, ones_f, den)
                recip = rcp.tile([1, 512], f32, tag="rc", name="rc")
                nc.vector.reciprocal_approx_fast(out=recip, in_=den_ps)
                rb = rbp.tile([128, 512], f32, tag="rb", name="rb")
                nc.gpsimd.partition_broadcast(rb, recip)
                nc.vector.tensor_mul(o_alls[qj][:, h, :], o_ps, rb)

            def outproj_unit(qj, e, o_all, split_q=False):
                """One output-feature chunk of the qj output projection."""
                op_ = ps_m.tile([128, 512], f32, tag="m", name="op")
                for h in range(8):
                    nc.tensor.matmul(
                        op_, wo_sb[:, h, e * 128:(e + 1) * 128],
                        o_all[:, h, :],
                        start=(h == 0), stop=(h == 7),
                    )
                ob = obuf.tile([128, 512], f32, tag="ob", name="ob")
                nc.vector.tensor_copy(ob, op_)
                eng = nc.scalar if (split_q and e % 2) else nc.sync
                eng.dma_start(
                    out=out[e * 128:(e + 1) * 128,
                            qj * 512:(qj + 1) * 512],
                    in_=ob,
                )

            # qj descending (longest attention bodies first); the previous
            # qj's 16 outproj units are spread 2-per-slot through the next
            # qj's body/tail slots so softmax-tail latency hides under them
            pending_tail = None
            pending_units = []
            for qj in (3, 2, 1, 0):
                o_alls[qj] = osb.tile([128, 8, 512], bf16, tag="oa", name="oa")
                for h in range(8):
                    st = emit_body(h, qj)
                    if h == 0:
                        # transition slot: the pending tail is h7 of the
                        # previous qj, which this qj's units depend on — it
                        # must precede them or the in-order PE queue deadlocks
                        if pending_tail is not None:
                            emit_tail(*pending_tail)
                            pending_tail = None
                    # outproj units before the pending tail: the tail's
                    # den-matmul gets the units' PE time as extra slack
                    for _ in range(2):
                        if pending_units:
                            pending_units.pop(0)()
                    if pending_tail is not None:
                        emit_tail(*pending_tail)
                    pending_tail = st
                prev = qj
                pending_units = [
                    (lambda e=e, q=prev, oa=o_alls[prev]: outproj_unit(q, e, oa))
                    for e in range(16)
                ]
            emit_tail(*pending_tail)
            # final block: no more compute to hide behind, so alternate the
            # output DMAs across both HW queues
            for e in range(16):
                outproj_unit(0, e, o_alls[0], split_q=True)

    nc.compile()
    return nc


def _get_program():
    if "nc" not in _PROG:
        _PROG["nc"] = _build_program()
    return _PROG["nc"]


def kernel(x, wq, wk, wv, wo, rope_cos, rope_sin):
    import ml_dtypes
    from concourse.bass_utils import run_bass_kernel_spmd

    bf16 = ml_dtypes.bfloat16
    fp8 = ml_dtypes.float8_e4m3
    nc = _get_program()
    x = np.asarray(x, dtype=np.float32)
    wq = np.asarray(wq, dtype=np.float32)
    wk = np.asarray(wk, dtype=np.float32)
    wv = np.asarray(wv, dtype=np.float32)
    wo = np.asarray(wo, dtype=np.float32)
    rope_cos = np.asarray(rope_cos, dtype=np.float32)
    rope_sin = np.asarray(rope_sin, dtype=np.float32)

    # even/odd -> [evens | odds] permutation of each head's rows of wq/wk
    perm = np.concatenate([np.arange(0, HD, 2), np.arange(1, HD, 2)])
    wq_p = wq.reshape(N_HEAD, HD, C)[:, perm, :]
    wk_p = wk.reshape(N_KV_HEAD, HD, C)[:, perm, :]

    cosT = rope_cos.T  # [64, T]
    sinT = rope_sin.T
    cs2 = np.concatenate([cosT, cosT], axis=0).astype(bf16)
    sb2 = np.concatenate([-sinT, sinT], axis=0).astype(bf16)

    in_maps = []
    for core in range(N_CORES):
        b, g = core // 2, core % 2
        wq_g = wq_p[8 * g:8 * g + 8].reshape(1024, C)
        wk_g = wk_p[2 * g:2 * g + 2].reshape(256, C)
        wv_g = wv.reshape(N_KV_HEAD, HD, C)[2 * g:2 * g + 2].reshape(256, C)
        in_maps.append({
            "xT": np.ascontiguousarray(x[b].T).astype(bf16),
            "wqT": np.ascontiguousarray(wq_g.T).astype(bf16),
            "wkT": np.ascontiguousarray(wk_g.T).astype(bf16),
            "wvT": np.ascontiguousarray(wv_g.T).astype(bf16),
            "woT": np.ascontiguousarray(
                wo[:, 1024 * g:1024 * (g + 1)].T
            ).astype(bf16),
            "cs2": cs2,
            "sb2": sb2,
        })

    global _LAST_IN_MAPS
    _LAST_IN_MAPS = in_maps
    res = run_bass_kernel_spmd(nc, in_maps, list(range(N_CORES))).results
    out = np.empty((B, T, C), dtype=np.float32)
    for b in range(B):
        out[b] = (res[2 * b]["out"] + res[2 * b + 1]["out"]).T
    return out


# revision 52
# speedup vs baseline: 1.3770x; 1.3770x over previous
"""GQA attention kernel for Trainium2, 8-core SPMD.

Sharding: core c = 2*b + g handles batch b (of 4) and head-group g (of 2):
8 of 16 q-heads, 2 of 4 kv-heads.  Each core computes its partial
out^T = (attn_out @ wo_g^T)^T in transposed space; the host adds the two
group partials per batch and transposes back.

v2.1 design (vs the fp32r 3-phase baseline):
  - all matmul operands bf16 (FWL weight loads, single x pass, half DMA)
  - x, Q, K, V, weights SBUF-resident; no Qd/Od DRAM round trips
  - DMAs emitted in first-use order so the PE starts within ~5us
  - fused attention + output projection in one qj-major loop
  - score tiles ki-paired: one exp per [128,2,512] PSUM region, den tree
    on [128,1024] pair tiles
  - softmax tail (den-reduce, 1/den, partition_broadcast, normalize)
    software-pipelined one head behind the s/PV matmuls so the in-order
    PE queue never stalls on DVE/Act work
  - output projection written to DRAM directly from PSUM (no Act copy)
Everything on-chip is in transposed orientation:
  Q^T/K^T: [head_dim(part), T]  scores^T: [kt(part), qt]  O^T: [d(part), qt]
RoPE is handled by permuting wq/wk rows on the host to an [evens | odds]
layout (scores are invariant to a shared d-permutation).
"""

import math
import numpy as np

B, T, C = 4, 2048, 2048
N_HEAD, N_KV_HEAD, HD = 16, 4, 128
N_CORES = 8
SCALE = 1.0 / math.sqrt(HD)

_PROG = {}
_LAST_IN_MAPS = None


def _build_program():
    from contextlib import ExitStack
    import concourse.bacc as bacc
    import concourse.mybir as mybir
    import concourse.tile as tile

    f32 = mybir.dt.float32
    bf16 = mybir.dt.bfloat16
    fp8 = mybir.dt.float8e4
    DR = mybir.MatmulPerfMode.DoubleRow
    Exp = mybir.ActivationFunctionType.Exp

    nc = bacc.Bacc(None, target_bir_lowering=False)
    xT = nc.declare_dram_parameter("xT", [C, T], bf16, isOutput=False)
    wqT = nc.declare_dram_parameter("wqT", [C, 1024], bf16, isOutput=False)
    wkT = nc.declare_dram_parameter("wkT", [C, 256], bf16, isOutput=False)
    wvT = nc.declare_dram_parameter("wvT", [C, 256], bf16, isOutput=False)
    woT = nc.declare_dram_parameter("woT", [1024, T], bf16, isOutput=False)
    cs2D = nc.declare_dram_parameter("cs2", [128, T], bf16, isOutput=False)
    sb2D = nc.declare_dram_parameter("sb2", [128, T], bf16, isOutput=False)
    out = nc.declare_dram_parameter("out", [C, T], f32, isOutput=True)

    with tile.TileContext(nc) as tc, nc.allow_low_precision(
        reason="bf16 operands validated end-to-end against 2e-2 rel-err gate"
    ), ExitStack() as top:
        consts = top.enter_context(tc.tile_pool(name="consts", bufs=1))
        cs2 = consts.tile([128, T], bf16)
        sb2 = consts.tile([128, T], bf16)
        ones_f = consts.tile([128, 1], bf16)
        nc.vector.memset(ones_f, 1.0)

        persist = top.enter_context(tc.tile_pool(name="persist", bufs=1))
        Q_sb = persist.tile([128, 8, T], bf16)
        K_sb = persist.tile([128, 2, T], bf16)
        V_sb = persist.tile([128, 16, 256], bf16)

        # ---- phase 1: QKV projections + RoPE, everything stays in SBUF ----
        with ExitStack() as ph1:
            xp = ph1.enter_context(tc.tile_pool(name="xp", bufs=1))
            x_sb = xp.tile([128, 16, T], bf16)
            wp = ph1.enter_context(tc.tile_pool(name="wp", bufs=1))
            wk_sb = wp.tile([128, 16, 256], bf16)
            wv_sb = wp.tile([128, 16, 256], bf16)
            wq_sb = wp.tile([128, 16, 1024], bf16)
            # Bulk loads all on the SP queue in PE-consumption order (the
            # DMA fabric is bandwidth-limited early on, so splitting queues
            # only reorders arrivals against the in-order consumer).
            nc.sync.dma_start(
                out=wk_sb, in_=wkT.rearrange("(n p) m -> p n m", p=128)
            )
            for ci in range(16):
                nc.sync.dma_start(
                    out=x_sb[:, ci, :],
                    in_=xT[ci * 128:(ci + 1) * 128, :],
                )
            nc.sync.dma_start(out=cs2, in_=cs2D[:])
            nc.sync.dma_start(out=sb2, in_=sb2D[:])
            nc.sync.dma_start(
                out=wv_sb, in_=wvT.rearrange("(n p) m -> p n m", p=128)
            )
            nc.sync.dma_start(
                out=wq_sb, in_=wqT.rearrange("(n p) m -> p n m", p=128)
            )

            raws = ph1.enter_context(tc.tile_pool(name="raws", bufs=4))
            tatb = ph1.enter_context(tc.tile_pool(name="tatb", bufs=4))
            pj_ps = ph1.enter_context(
                tc.tile_pool(name="pj_ps", bufs=3, space="PSUM")
            )

            def proj_half(w_sb, fsl, hb):
                """Project one 128-feature block over a 1024-token half.
                Returns the raw (pre-RoPE) bf16 SBUF tile [128, 1024]."""
                hsl = slice(hb * 1024, (hb + 1) * 1024)
                ps = pj_ps.tile([128, 2, 512], f32, tag="pj", name="pj")
                for ci in range(16):
                    for c2 in range(2):
                        t0 = hb * 1024 + c2 * 512
                        nc.tensor.matmul(
                            ps[:, c2, :],
                            w_sb[:, ci, fsl],
                            x_sb[:, ci, t0:t0 + 512],
                            start=(ci == 0), stop=(ci == 15),
                        )
                raw = raws.tile([128, 1024], bf16, tag="raw", name="raw")
                for c2 in range(2):
                    nc.vector.tensor_copy(
                        raw[:, c2 * 512:(c2 + 1) * 512], ps[:, c2, :]
                    )
                return raw, hsl

            def rope_half(raw, hsl, dest):
                """dest[:, hsl] = raw * cs2 + swap(raw) * sb2.
                The half-swap along the partition (head-dim) axis is done
                with two SBUF->SBUF DMAs instead of a permutation matmul."""
                swr = raws.tile([128, 1024], bf16, tag="swr", name="swr")
                nc.sync.dma_start(out=swr[0:64, :], in_=raw[64:128, :])
                nc.sync.dma_start(out=swr[64:128, :], in_=raw[0:64, :])
                ta = tatb.tile([128, 1024], bf16, tag="ta", name="ta")
                tb = tatb.tile([128, 1024], bf16, tag="tb", name="tb")
                nc.vector.tensor_mul(ta, raw, cs2[:, hsl])
                nc.vector.tensor_mul(tb, swr, sb2[:, hsl])
                nc.vector.tensor_add(dest, ta, tb)

            # K (2 kv heads, rope'd) first so attention deps resolve early
            for kv in range(2):
                for hb in range(2):
                    raw, hsl = proj_half(wk_sb, slice(kv * 128, (kv + 1) * 128), hb)
                    rope_half(raw, hsl, K_sb[:, kv, hsl])
            # V (2 kv heads = 2 d-chunks), transposed into [t, d] layout
            # via the DMA XBAR instead of PE transposes
            for dv in range(2):
                for hb in range(2):
                    raw, hsl = proj_half(wv_sb, slice(dv * 128, (dv + 1) * 128), hb)
                    for k8 in range(8):
                        ki = hb * 8 + k8
                        nc.sync.dma_start_transpose(
                            out=V_sb[:, ki, dv * 128:(dv + 1) * 128],
                            in_=raw[:, k8 * 128:(k8 + 1) * 128],
                        )
            # Q (8 heads, rope'd)
            for h in range(8):
                for hb in range(2):
                    raw, hsl = proj_half(
                        wq_sb, slice(h * 128, (h + 1) * 128), hb
                    )
                    rope_half(raw, hsl, Q_sb[:, h, hsl])

        # ---- phase 2: causal attention fused with output projection ----
        with ExitStack() as ph2:
            wop = ph2.enter_context(tc.tile_pool(name="wop", bufs=1))
            wo_sb = wop.tile([128, 8, T], bf16)
            nc.sync.dma_start(
                out=wo_sb, in_=woT.rearrange("(h p) e -> p h e", p=128)
            )
            p_pool = ph2.enter_context(tc.tile_pool(name="p_pool", bufs=20))
            dtree = ph2.enter_context(tc.tile_pool(name="dtree", bufs=12))
            dn128 = ph2.enter_context(tc.tile_pool(name="dn128", bufs=2))
            rcp = ph2.enter_context(tc.tile_pool(name="rcp", bufs=2))
            rbp = ph2.enter_context(tc.tile_pool(name="rbp", bufs=2))
            osb = ph2.enter_context(tc.tile_pool(name="osb", bufs=2))
            obuf = ph2.enter_context(tc.tile_pool(name="obuf", bufs=4))
            ps_s = ph2.enter_context(
                tc.tile_pool(name="ps_s", bufs=2, space="PSUM")
            )
            ps_o = ph2.enter_context(
                tc.tile_pool(name="ps_o", bufs=2, space="PSUM")
            )
            ps_m = ph2.enter_context(
                tc.tile_pool(name="ps_m", bufs=2, space="PSUM")
            )

            o_alls = {}

            def emit_body(h, qj):
                """s/exp/mask/PV matmuls for one (head, q-tile).
                Score tiles are processed in ki pairs: 2 s-matmuls into one
                [128,2,512] PSUM region, a single exp over both, then 2 PV
                accumulation matmuls.  Diagonal-tile matmuls are narrowed to
                the causally live columns; exp/select stay full-width so the
                skipped (stale) columns are forced to exact zero."""
                kv = h // 4
                nk = 4 * (qj + 1)
                o_ps = ps_o.tile([128, 512], f32, name="o_ps")
                pairs = []
                for kp in range(nk // 2):
                    s_ps = ps_s.tile([128, 2, 512], f32, name="s_ps")
                    for j in range(2):
                        ki = 2 * kp + j
                        lo = max(0, ki * 128 - qj * 512)
                        nc.tensor.matmul(
                            s_ps[:, j, lo:],
                            K_sb[:, kv, ki * 128:(ki + 1) * 128],
                            Q_sb[:, h, qj * 512 + lo:(qj + 1) * 512],
                        )
                    pp = p_pool.tile([128, 2, 512], bf16, tag="p", name="p")
                    # exp narrowed to causally-live columns of the pair; the
                    # full-width selects below zero everything to the left
                    lo0 = max(0, 2 * kp * 128 - qj * 512)
                    nc.scalar.activation(
                        pp[:, :, lo0:], s_ps[:, :, lo0:], Exp, scale=SCALE
                    )
                    for j in range(2):
                        ki = 2 * kp + j
                        if ki >= 4 * qj:
                            nc.gpsimd.affine_select(
                                out=pp[:, j, :], in_=pp[:, j, :],
                                pattern=[[1, 512]],
                                compare_op=mybir.AluOpType.is_ge, fill=0.0,
                                base=qj * 512 - ki * 128,
                                channel_multiplier=-1,
                            )
                    for j in range(2):
                        ki = 2 * kp + j
                        lo = max(0, ki * 128 - qj * 512)
                        nc.tensor.matmul(
                            o_ps[:, lo:],
                            V_sb[:, ki, kv * 128:(kv + 1) * 128],
                            pp[:, j, lo:],
                            start=(ki == 0), stop=(ki == nk - 1),
                        )
                    pairs.append(pp)
                return h, qj, pairs, o_ps

            def emit_tail(h, qj, pairs, o_ps):
                """den reduce + 1/den + normalize into O_all[qj][:, h, :]."""
                # pairwise bf16 tree over [128,1024] pair tiles
                lvl = pairs
                while len(lvl) > 1:
                    nxt = []
                    for i in range(0, len(lvl) - 1, 2):
                        t = dtree.tile([128, 2, 512], bf16, tag="dt", name="dt")
                        nc.vector.tensor_add(t, lvl[i], lvl[i + 1])
                        nxt.append(t)
                    if len(lvl) % 2:
                        nxt.append(lvl[-1])
                    lvl = nxt
                den = dn128.tile([128, 512], bf16, tag="dn", name="dn")
                nc.vector.tensor_add(den, lvl[0][:, 0, :], lvl[0][:, 1, :])
                den_ps = ps_m.tile([1, 512], f32, tag="m", name="dps")
                nc.tensor.matmul(den_ps, ones_f, den)
                recip = rcp.tile([1, 512], f32, tag="rc", name="rc")
                nc.vector.reciprocal_approx_fast(out=recip, in_=den_ps)
                rb = rbp.tile([128, 512], f32, tag="rb", name="rb")
                nc.gpsimd.partition_broadcast(rb, recip)
                nc.vector.tensor_mul(o_alls[qj][:, h, :], o_ps, rb)

            def outproj_unit(qj, e, o_all, split_q=False):
                """One output-feature chunk of the qj output projection."""
                op_ = ps_m.tile([128, 512], f32, tag="m", name="op")
                for h in range(8):
                    nc.tensor.matmul(
                        op_, wo_sb[:, h, e * 128:(e + 1) * 128],
                        o_all[:, h, :],
                        start=(h == 0), stop=(h == 7),
                    )
                ob = obuf.tile([128, 512], f32, tag="ob", name="ob")
                nc.vector.tensor_copy(ob, op_)
                eng = nc.scalar if (split_q and e % 2) else nc.sync
                eng.dma_start(
                    out=out[e * 128:(e + 1) * 128,
                            qj * 512:(qj + 1) * 512],
                    in_=ob,
                )

            # qj descending (longest attention bodies first); the previous
            # qj's 16 outproj units are spread 2-per-slot through the next
            # qj's body/tail slots so softmax-tail latency hides under them
            pending_tail = None
            pending_units = []
            for qj in (3, 2, 1, 0):
                o_alls[qj] = osb.tile([128, 8, 512], bf16, tag="oa", name="oa")
                for h in range(8):
                    st = emit_body(h, qj)
                    if pending_tail is not None:
                        emit_tail(*pending_tail)
                    pending_tail = st
                    for _ in range(2):
                        if pending_units:
                            pending_units.pop(0)()
                prev = qj
                pending_units = [
                    (lambda e=e, q=prev, oa=o_alls[prev]: outproj_unit(q, e, oa))
                    for e in range(16)
                ]
            emit_tail(*pending_tail)
            # final block: no more compute to hide behind, so alternate the
            # output DMAs across both HW queues
            for e in range(16):
                outproj_unit(0, e, o_alls[0], split_q=True)

    nc.compile()
    return nc


def _get_program():
    if "nc" not in _PROG:
        _PROG["nc"] = _build_program()
    return _PROG["nc"]


def kernel(x, wq, wk, wv, wo, rope_cos, rope_sin):
    import ml_dtypes
    from concourse.bass_utils import run_bass_kernel_spmd

    bf16 = ml_dtypes.bfloat16
    fp8 = ml_dtypes.float8_e4m3
    nc = _get_program()
    x = np.asarray(x, dtype=np.float32)
    wq = np.asarray(wq, dtype=np.float32)
    wk = np.asarray(wk, dtype=np.float32)
    wv = np.asarray(wv, dtype=np.float32)
    wo = np.asarray(wo, dtype=np.float32)
    rope_cos = np.asarray(rope_cos, dtype=np.float32)
    rope_sin = np.asarray(rope_sin, dtype=np.float32)

    # even/odd -> [evens | odds] permutation of each head's rows of wq/wk
    perm = np.concatenate([np.arange(0, HD, 2), np.arange(1, HD, 2)])
    wq_p = wq.reshape(N_HEAD, HD, C)[:, perm, :]
    wk_p = wk.reshape(N_KV_HEAD, HD, C)[:, perm, :]

    cosT = rope_cos.T  # [64, T]
    sinT = rope_sin.T
    cs2 = np.concatenate([cosT, cosT], axis=0).astype(bf16)
    sb2 = np.concatenate([-sinT, sinT], axis=0).astype(bf16)

    in_maps = []
    for core in range(N_CORES):
        b, g = core // 2, core % 2
        wq_g = wq_p[8 * g:8 * g + 8].reshape(1024, C)
        wk_g = wk_p[2 * g:2 * g + 2].reshape(256, C)
        wv_g = wv.reshape(N_KV_HEAD, HD, C)[2 * g:2 * g + 2].reshape(256, C)
        in_maps.append({
            "xT": np.ascontiguousarray(x[b].T).astype(bf16),
            "wqT": np.ascontiguousarray(wq_g.T).astype(bf16),
            "wkT": np.ascontiguousarray(wk_g.T).astype(bf16),
            "wvT": np.ascontiguousarray(wv_g.T).astype(bf16),
            "woT": np.ascontiguousarray(
                wo[:, 1024 * g:1024 * (g + 1)].T
            ).astype(bf16),
            "cs2": cs2,
            "sb2": sb2,
        })

    global _LAST_IN_MAPS
    _LAST_IN_MAPS = in_maps
    res = run_bass_kernel_spmd(nc, in_maps, list(range(N_CORES))).results
    out = np.empty((B, T, C), dtype=np.float32)
    for b in range(B):
        out[b] = (res[2 * b]["out"] + res[2 * b + 1]["out"]).T
    return out
